# revision 4
# baseline (speedup 1.0000x reference)
"""DiffuseRouter kernel for 8 TRN2 NeuronCores.

Reference computation (enable_time=False, soft_time_routing=True):
    out[b, l, d] = (1/3) * sum_g sum_e expert_emb_g[e, b, l, d]
i.e. a uniform-weighted sum of 28 expert planes per batch element.

Sharding: pure data-parallel over batch B=8 -> one batch element per core.
Each core reads its 28 [256, 1280] f32 planes (36.7 MB), reduces them
on-chip, scales by 1/3, and writes its [256, 1280] output.  No collectives
needed (B == n_cores).

Engine assignment (v2): the 28-plane reduction runs on the *TensorE* as
identity matmuls accumulating into 5 PSUM banks (fp32r moving operand:
1 cycle/row, ~213 ns per 512-col bank, ~30 us total) instead of a serial
DVE scalar_tensor_tensor chain (fp32 stt is capped at 1x perf mode,
~77 us, which trailed the 425 GB/s DMA stream by ~25 us).  The final
x1/3 scale runs on the otherwise-idle ACT engine straight out of PSUM,
per bank, so stores pipeline behind the last plane's chunk loads.
fp32r rounds each product input to ~bf16 mantissa: expected rel err
~1e-3 on this sum-of-28-randn workload (gate is 2e-2).

All SBUF input tiles are declared float32r (same 4-byte bits as f32; the
DRAM-side APs are bitcast) because the BIR verifier requires fp32r
matmul operands to come from fp32r-typed producers.
"""

import numpy as np

import concourse.bacc as bacc
import concourse.tile as tile
from concourse import mybir
from concourse.bass_utils import run_bass_kernel_spmd

N_CORES = 8
E_TOTAL = 28  # 4 + 8 + 16 experts across the 3 granularity levels
L, D = 256, 1280
P = 128  # SBUF partitions
FD = (L // P) * D  # 2560 free-dim elements per partition
NB = 5  # PSUM banks used
BW = FD // NB  # 512 f32 per bank = exactly one 2 KB PSUM bank
SCALE = 1.0 / 3.0

_NC_CACHE = None


def _build_nc():
    """Build the SPMD Bass program (identical on all 8 cores).

    Loads stream the 28 expert planes as [128, 2560] tiles (1.31 MB linear
    DMAs) on the SP HWDGE ring.  TensorE accumulates each plane into PSUM
    via matmul with a stationary fp32r identity (psum[b] += I.T @ chunk);
    start=True on plane 0 clears the banks, stop=True on plane 27 closes
    the group.  The last plane is loaded as 5 separate bank-chunk DMAs so
    each bank's final matmul -> ACT scale -> store fires as soon as its
    own 0.26 MB chunk lands, keeping the post-stream tail to ~3 us.
    """
    nc = bacc.Bacc(
        "TRN2", target_bir_lowering=False, debug=False, enable_partition_id=False
    )
    x = nc.dram_tensor("x", [E_TOTAL, L, D], mybir.dt.float32, kind="ExternalInput")
    ident_d = nc.dram_tensor("ident", [P, P], mybir.dt.float32, kind="ExternalInput")
    out = nc.dram_tensor("out", [L, D], mybir.dt.float32, kind="ExternalOutput")

    # [E, 256, 1280] -> [E, 128, 2560]: partition p holds rows 2p, 2p+1
    # (contiguous 10240 B per partition -> fully linear 1.31 MB DMA per plane).
    f32 = mybir.dt.float32
    f32r = mybir.dt.float32r
    x_t = x.ap().rearrange("e (p a) d -> e p (a d)", a=2).bitcast(f32r)
    out_t = out.ap().rearrange("(p a) d -> p (a d)", a=2)

    with tile.TileContext(nc) as tc:
        with (
            tc.tile_pool(name="in", bufs=8) as pin,
            tc.tile_pool(name="const", bufs=1) as pconst,
            tc.tile_pool(name="out", bufs=1) as pout,
            tc.tile_pool(name="ps", bufs=1, space="PSUM") as pps,
        ):
            ident = pconst.tile([P, P], f32r, name="ident", tag="ident")
            # Identity comes in from DRAM on the ACT ring so the SP ring
            # carries nothing but the 28 plane loads.
            nc.scalar.dma_start(out=ident[:], in_=ident_d.ap().bitcast(f32r))
            psums = [
                pps.tile([P, BW], f32, name=f"ps{b}", tag=f"ps{b}") for b in range(NB)
            ]
            outs = pout.tile([P, FD], f32, name="outs", tag="outs")

            last = E_TOTAL - 1
            for e in range(E_TOTAL):
                if e < last:
                    t = pin.tile([P, FD], f32r)
                    nc.sync.dma_start(out=t[:], in_=x_t[e])
                    chunks = [t[:, b * BW : (b + 1) * BW] for b in range(NB)]
                else:
                    # Last plane: five bank-chunk loads in separate tiles so
                    # each bank's closing matmul starts as soon as its own
                    # chunk lands (not the whole plane).
                    chunks = []
                    for b in range(NB):
                        ct = pin.tile([P, BW], f32r, name=f"c{b}", tag=f"c{b}")
                        nc.sync.dma_start(
                            out=ct[:], in_=x_t[e][:, b * BW : (b + 1) * BW]
                        )
                        chunks.append(ct[:])
                for b in range(NB):
                    # psum[b] (+)= chunk  via  I.T @ chunk, fp32r single-pass.
                    nc.tensor.matmul(
                        psums[b][:],
                        ident[:],
                        chunks[b],
                        start=(e == 0),
                        stop=(e == last),
                    )
                    if e == last:
                        bs = slice(b * BW, (b + 1) * BW)
                        # ACT: out = psum * 1/3 (PSUM -> SBUF), then store on
                        # the ACT HWDGE ring (SP ring is busy with loads).
                        nc.scalar.mul(outs[:, bs], psums[b][:], SCALE)
                        nc.scalar.dma_start(out=out_t[:, bs], in_=outs[:, bs])
    nc.compile()
    return nc


def _get_nc():
    global _NC_CACHE
    if _NC_CACHE is None:
        _NC_CACHE = _build_nc()
    return _NC_CACHE


def _run(inputs, trace=False, trace_kwargs=None):
    e0 = np.asarray(inputs["expert_emb_0"], dtype=np.float32)
    e1 = np.asarray(inputs["expert_emb_1"], dtype=np.float32)
    e2 = np.asarray(inputs["expert_emb_2"], dtype=np.float32)
    B = e0.shape[1]
    assert B == N_CORES, f"expected B == {N_CORES}, got {B}"

    ident = np.eye(P, dtype=np.float32)
    in_maps = []
    for b in range(B):
        xb = np.concatenate([e0[:, b], e1[:, b], e2[:, b]], axis=0)
        in_maps.append({"x": np.ascontiguousarray(xb), "ident": ident})

    kw = {}
    if trace:
        kw["trace"] = True
        if trace_kwargs:
            kw.update(trace_kwargs)
    try:
        res = run_bass_kernel_spmd(_get_nc(), in_maps, list(range(N_CORES)), **kw)
    except Exception:
        # One retry: transient device errors (e.g. NRT unrecoverable after a
        # prior wedged run) usually clear on re-dispatch.
        res = run_bass_kernel_spmd(_get_nc(), in_maps, list(range(N_CORES)), **kw)
    out = np.stack([res.results[b]["out"] for b in range(B)], axis=0)
    return out.astype(np.float32, copy=False), res


def kernel(**inputs) -> np.ndarray:
    out, _ = _run(inputs, trace=False)
    return out


# revision 5
# speedup vs baseline: 1.0498x; 1.0498x over previous
"""DiffuseRouter kernel for 8 TRN2 NeuronCores.

Reference computation (enable_time=False, soft_time_routing=True):
    out[b, l, d] = (1/3) * sum_g sum_e expert_emb_g[e, b, l, d]
i.e. a uniform-weighted sum of 28 expert planes per batch element.

Sharding: pure data-parallel over batch B=8 -> one batch element per core.
Each core reads its 28 [256, 1280] f32 planes (36.7 MB), reduces them
on-chip, scales by 1/3, and writes its [256, 1280] output.  No collectives
needed (B == n_cores).

Engine assignment (v3): the DMA stream sustains ~425 GB/s (3.05 us per
plane), so the reduction is split across two engines that each keep pace:

  * TensorE sums free-dim columns [0, 1536) via identity matmuls
    accumulating into 3 PSUM banks (fp32r moving operand, 1 cycle/row;
    fp32r never leaves the 1.2 GHz MID clock, so a full 5-bank PE
    version at ~3.9 us/plane would throttle the stream -- 3 banks run
    at ~2.4 us/plane).  ACT applies the final x1/3 from PSUM per bank.
  * DVE sums columns [1536, 2560) with a scalar_tensor_tensor chain
    (fp32 1x mode, ~1.2 us/plane) with the 1/3 scale folded in.

The last plane is loaded as 5 bank-chunk DMAs (PE's chunks first, DVE's
last) so each column range's final op -> store fires as soon as its own
0.26 MB chunk lands, keeping the post-stream tail to ~3 us.
fp32r rounds matmul inputs to ~19-bit mantissa: measured rel err ~1e-4
(gate is 2e-2).
"""

import numpy as np

import concourse.bacc as bacc
import concourse.tile as tile
from concourse import mybir
from concourse.alu_op_type import AluOpType
from concourse.bass_utils import run_bass_kernel_spmd

N_CORES = 8
E_TOTAL = 28  # 4 + 8 + 16 experts across the 3 granularity levels
L, D = 256, 1280
P = 128  # SBUF partitions
FD = (L // P) * D  # 2560 free-dim elements per partition
BW = 512  # one 2 KB PSUM bank of f32
NB_PE = 3  # banks summed on TensorE (cols 0..1536)
DVE_LO = NB_PE * BW  # 1536: start of the DVE column range
DVE_W = FD - DVE_LO  # 1024 cols summed on DVE
SCALE = 1.0 / 3.0

_NC_CACHE = None


def _build_nc():
    """Build the SPMD Bass program (identical on all 8 cores)."""
    nc = bacc.Bacc(
        "TRN2", target_bir_lowering=False, debug=False, enable_partition_id=False
    )
    x = nc.dram_tensor("x", [E_TOTAL, L, D], mybir.dt.float32, kind="ExternalInput")
    ident_d = nc.dram_tensor("ident", [P, P], mybir.dt.float32, kind="ExternalInput")
    out = nc.dram_tensor("out", [L, D], mybir.dt.float32, kind="ExternalOutput")

    # [E, 256, 1280] -> [E, 128, 2560]: partition p holds rows 2p, 2p+1
    # (contiguous 10240 B per partition -> fully linear 1.31 MB DMA per plane).
    f32 = mybir.dt.float32
    f32r = mybir.dt.float32r
    x_t = x.ap().rearrange("e (p a) d -> e p (a d)", a=2)
    x_tr = x_t.bitcast(f32r)
    out_t = out.ap().rearrange("(p a) d -> p (a d)", a=2)

    mult = AluOpType.mult
    add = AluOpType.add

    with tile.TileContext(nc) as tc:
        with (
            tc.tile_pool(name="in", bufs=8) as pin,
            tc.tile_pool(name="const", bufs=1) as pconst,
            tc.tile_pool(name="acc", bufs=1) as pacc,
            tc.tile_pool(name="ps", bufs=1, space="PSUM") as pps,
        ):
            ident = pconst.tile([P, P], f32r, name="ident", tag="ident")
            # Identity comes in from DRAM on the ACT ring so the SP ring
            # carries nothing but the 28 plane loads.
            nc.scalar.dma_start(out=ident[:], in_=ident_d.ap().bitcast(f32r))
            psums = [
                pps.tile([P, BW], f32, name=f"ps{b}", tag=f"ps{b}")
                for b in range(NB_PE)
            ]
            # ACT staging for the PE banks' scaled output.
            outs = pacc.tile([P, NB_PE * BW], f32, name="outs", tag="outs")
            # DVE accumulator for cols [1536, 2560), scale folded into adds.
            acc = pacc.tile([P, DVE_W], f32, name="acc", tag="acc")

            last = E_TOTAL - 1
            for e in range(E_TOTAL):
                if e < last:
                    # One linear 1.31 MB load per plane; PE reads the f32r
                    # view, DVE reads the same bytes bitcast back to f32.
                    t = pin.tile([P, FD], f32r)
                    nc.sync.dma_start(out=t[:], in_=x_tr[e])
                    pe_chunks = [t[:, b * BW : (b + 1) * BW] for b in range(NB_PE)]
                    dve_chunks = [t[:, DVE_LO:FD].bitcast(f32)]
                else:
                    # Last plane: five bank-chunk loads in separate tiles so
                    # each column range's final op starts as soon as its own
                    # chunk lands.  PE chunks load first, DVE chunks last
                    # (the DVE tail per chunk is shorter).
                    pe_chunks = []
                    for b in range(NB_PE):
                        ct = pin.tile([P, BW], f32r, name=f"c{b}", tag=f"c{b}")
                        nc.sync.dma_start(
                            out=ct[:], in_=x_tr[e][:, b * BW : (b + 1) * BW]
                        )
                        pe_chunks.append(ct[:])
                    dve_chunks = []
                    for q in range(2):
                        lo = DVE_LO + q * BW
                        ct = pin.tile([P, BW], f32, name=f"d{q}", tag=f"d{q}")
                        nc.sync.dma_start(out=ct[:], in_=x_t[e][:, lo : lo + BW])
                        dve_chunks.append(ct[:])

                for b in range(NB_PE):
                    # psum[b] (+)= chunk  via  I.T @ chunk, fp32r single-pass.
                    nc.tensor.matmul(
                        psums[b][:],
                        ident[:],
                        pe_chunks[b],
                        start=(e == 0),
                        stop=(e == last),
                    )
                    if e == last:
                        bs = slice(b * BW, (b + 1) * BW)
                        # ACT: out = psum * 1/3 (PSUM -> SBUF), then store on
                        # the ACT HWDGE ring (SP ring is busy with loads).
                        nc.scalar.mul(outs[:, bs], psums[b][:], SCALE)
                        nc.scalar.dma_start(out=out_t[:, bs], in_=outs[:, bs])

                if e == 0:
                    # acc = t0 * 1/3 (tensor_scalar: 2x perf mode)
                    nc.vector.tensor_scalar_mul(acc[:], dve_chunks[0], SCALE)
                elif e < last:
                    # acc = (t_e * 1/3) + acc
                    nc.vector.scalar_tensor_tensor(
                        acc[:], dve_chunks[0], SCALE, acc[:], mult, add
                    )
                else:
                    for q in range(2):
                        qs = slice(q * BW, (q + 1) * BW)
                        nc.vector.scalar_tensor_tensor(
                            acc[:, qs], dve_chunks[q], SCALE, acc[:, qs], mult, add
                        )
                        nc.scalar.dma_start(
                            out=out_t[:, DVE_LO + q * BW : DVE_LO + (q + 1) * BW],
                            in_=acc[:, qs],
                        )
    nc.compile()
    return nc


def _get_nc():
    global _NC_CACHE
    if _NC_CACHE is None:
        _NC_CACHE = _build_nc()
    return _NC_CACHE


def _run(inputs, trace=False, trace_kwargs=None):
    e0 = np.asarray(inputs["expert_emb_0"], dtype=np.float32)
    e1 = np.asarray(inputs["expert_emb_1"], dtype=np.float32)
    e2 = np.asarray(inputs["expert_emb_2"], dtype=np.float32)
    B = e0.shape[1]
    assert B == N_CORES, f"expected B == {N_CORES}, got {B}"

    ident = np.eye(P, dtype=np.float32)
    in_maps = []
    for b in range(B):
        xb = np.concatenate([e0[:, b], e1[:, b], e2[:, b]], axis=0)
        in_maps.append({"x": np.ascontiguousarray(xb), "ident": ident})

    kw = {}
    if trace:
        kw["trace"] = True
        if trace_kwargs:
            kw.update(trace_kwargs)
    try:
        res = run_bass_kernel_spmd(_get_nc(), in_maps, list(range(N_CORES)), **kw)
    except Exception:
        # One retry: transient device errors (e.g. NRT unrecoverable after a
        # prior wedged run) usually clear on re-dispatch.
        res = run_bass_kernel_spmd(_get_nc(), in_maps, list(range(N_CORES)), **kw)
    out = np.stack([res.results[b]["out"] for b in range(B)], axis=0)
    return out.astype(np.float32, copy=False), res


def kernel(**inputs) -> np.ndarray:
    out, _ = _run(inputs, trace=False)
    return out
